# revision 37
# baseline (speedup 1.0000x reference)
"""Causal self-attention TRN2 Bass kernel (phase-interleaved).

Problem: B=4, T=2048, C=1024, H=16 heads (HD=64), torch-Linear semantics
(y = x @ W.T + b), causal + padding mask, softmax, output projection.

Sharding: 8 cores = (batch b in 0..3) x (head-half in 0..1). Each core
handles one batch and 8 heads (512 of the 1024 channels of QKV / of the
contraction dim of the output projection). The two half-cores of a batch
produce partial output projections that the host sums (plus bp).

Design (376us baseline -> ~276us):
  - Attention q-chunks run ASCENDING; PE-bound projection and
    output-projection units are interleaved as filler between the
    ACT-bound softmax steps so the Tensor engine never idles while ACT
    grinds exp(). Fill is placed by measured engine balance: rounds 1-2
    are PE-bound (proj fill only); round 3 is ACT-bound with ~17us of
    PE slack, so ALL output-projection fill (chunks 1 and 2) parks
    there:
      proj(0) -> [attn(0)|proj(1)] -> [attn(1)|proj(2)|out(0)]
              -> [attn(2)|proj(3)] -> [attn(3)|out(1)|out(2)] -> out(3)
  - All inputs shipped bf16 (same matmul rate, half the DMA bytes);
    V and exp(S) tiles bf16 (faster LDWEIGHTS, 2x DVE mask multiply).
  - DMA issue spread across the 3 capable queues (sync/scalar/gpsimd),
    ~700ns per dma_start per queue; first x/Wq tiles split in half so
    the first matmul starts ~10us in.
  - Causal diag mask: DVE multiply with a 0/1 triangle after exp
    (no PE identity-matmuls); V bias added during the DVE psum drain.
  - Rowsum via a ones-column in the V stationary ([V|1]); normalization:
    psum row -> SBUF copy -> reciprocal_approx_fast -> partition
    broadcast -> DVE multiply (approx-recip cannot read PSUM directly).
  - Y^T kept as four per-head-group tiles so output-projection reads
    depend only on the epilogue that wrote them; the last four token
    tiles use a 6-buffer psum pool (attention pools released first) so
    their partial accumulations cover the final epilogue's latency.
  - PSUM: 2 proj/out + 2x2 S + 1x2 O banks = 8 during attention.
  - Engine budget: PE ~245us (the roofline), ACT ~158us of exp,
    DVE ~130us of drains/masks/normalize, sync ~45us of DMA issue.
"""

import ml_dtypes
import numpy as np

import concourse.mybir as mybir
import concourse.tile as tile
from concourse import bacc
from concourse.bass_utils import run_bass_kernel_spmd

F32 = mybir.dt.float32
F32R = mybir.dt.float32r
BF16 = mybir.dt.bfloat16
F8 = mybir.dt.float8e4
DR = mybir.MatmulPerfMode.DoubleRow
AF = mybir.ActivationFunctionType
ALU = mybir.AluOpType

B, T, C, H = 4, 2048, 1024, 16
HD = C // H          # 64
IC = C // 2          # 512 channels per core (8 heads)
NKT = T // 128       # 16 k-tiles
NCT = C // 128       # 8 contraction tiles for QKV
NEG = -1.0e30
SCALE = 1.0 / np.sqrt(HD)
D = 8                # S->AV pipeline lag (in k-tile steps)

_CACHE = {}


def _build():
    nc = bacc.Bacc("TRN2", target_bir_lowering=False, debug=False)

    xT_d = nc.dram_tensor("xT", [C, T], BF16, kind="ExternalInput").ap()
    xT8_d = nc.dram_tensor("xT8", [C, T], F8, kind="ExternalInput").ap()
    WqT8_d = nc.dram_tensor("WqT8", [C, IC], F8, kind="ExternalInput").ap()
    WkT8_d = nc.dram_tensor("WkT8", [C, IC], F8, kind="ExternalInput").ap()
    WvT8_d = nc.dram_tensor("WvT8", [C, IC], F8, kind="ExternalInput").ap()
    WqT_d = nc.dram_tensor("WqT", [C, IC], BF16, kind="ExternalInput").ap()
    WkT_d = nc.dram_tensor("WkT", [C, IC], BF16, kind="ExternalInput").ap()
    WvT_d = nc.dram_tensor("WvT", [C, IC], BF16, kind="ExternalInput").ap()
    WpT_d = nc.dram_tensor("WpT", [IC, C], BF16, kind="ExternalInput").ap()
    bq_d = nc.dram_tensor("bqs", [128, 4], F32, kind="ExternalInput").ap()
    bk_d = nc.dram_tensor("bks", [128, 4], F32, kind="ExternalInput").ap()
    bv_d = nc.dram_tensor("bvr", [1, IC], F32, kind="ExternalInput").ap()
    pad_d = nc.dram_tensor("padb", [128, NKT], F32, kind="ExternalInput").ap()
    tri_d = nc.dram_tensor("tri01", [128, 128], BF16, kind="ExternalInput").ap()
    idn_d = nc.dram_tensor("idn", [128, 128], BF16, kind="ExternalInput").ap()
    trn_d = nc.dram_tensor("trneg", [128, 512], BF16, kind="ExternalInput").ap()
    ones_d = nc.dram_tensor("ones8", [128, 8], BF16, kind="ExternalInput").ap()
    out_d = nc.dram_tensor("out", [C, T], BF16, kind="ExternalOutput").ap()

    with tile.TileContext(nc) as tc:
        with tc.tile_pool(name="pp", bufs=1) as pp:
            QT = pp.tile([128, 4 * T], BF16, name="QT")
            KT = pp.tile([128, 4 * T], BF16, name="KT")
            Vt = pp.tile([128, 4 * 520], BF16, name="Vt")
            Vt8 = pp.tile([128, NKT * 528], F8, name="Vt8")
            YTg = [pp.tile([128, T], BF16, name=f"YT{i}", uniquify=False)
                   for i in range(4)]
            Wp_sb = pp.tile([128, 4 * C], BF16, name="Wp_sb")
            Wq_sb = pp.tile([128, NCT * 512], BF16, name="Wq_sb")
            Wk_sb = pp.tile([128, NCT * 512], BF16, name="Wk_sb")
            Wv_sb = pp.tile([128, NCT * 512], BF16, name="Wv_sb")
            Wq8_sb = pp.tile([128, NCT * 512], F8, name="Wq8_sb")
            Wk8_sb = pp.tile([128, NCT * 512], F8, name="Wk8_sb")
            Wv8_sb = pp.tile([128, NCT * 512], F8, name="Wv8_sb")
            bq_sb = pp.tile([128, 4], F32, name="bq_sb")
            bk_sb = pp.tile([128, 4], F32, name="bk_sb")
            bv_sb = pp.tile([1, IC], F32, name="bv_sb")
            bvb_sb = pp.tile([128, IC], F32, name="bvb_sb")
            pad_sb = pp.tile([128, NKT], F32, name="pad_sb")
            tri_sb = pp.tile([128, 128], BF16, name="tri_sb")
            idn_sb = pp.tile([128, 128], BF16, name="idn_sb")
            trn_sb = pp.tile([128, 512], BF16, name="trn_sb")
            one8_sb = pp.tile([128, 8], BF16, name="one8_sb")

            # bf16 V for the first 4 k-tiles (qc=0); fp8 k-tile-paired V
            # (pair pr holds k-tiles 2pr, 2pr+1) for the DoubleRow AV.
            Vf = Vt.rearrange("p (k h c) -> p k h c", k=4, h=8, c=65)
            # c padded to 66 so the pair-dim (j) byte stride is 528,
            # a multiple of 16 as the DoubleRow LDWEIGHTS ISA requires
            Vf8 = Vt8.rearrange("p (pr j h c) -> p pr j h c",
                                pr=NKT // 2, j=2, h=8, c=66)
            nc.gpsimd.dma_start(out=one8_sb[:], in_=ones_d)
            for kt in range(4):
                nc.vector.tensor_copy(Vf[:, kt, :, 64], one8_sb[:, :, None])
            for pr in range(NKT // 2):
                for j in range(2):
                    nc.vector.tensor_copy(Vf8[:, pr, j, :, 64],
                                          one8_sb[:, :, None])

            xs = tc.alloc_tile_pool(name="xs", bufs=2)
            pss = tc.alloc_tile_pool(name="pss", bufs=2, space="PSUM")
            pso = tc.alloc_tile_pool(name="pso", bufs=3, space="PSUM")
            es = tc.alloc_tile_pool(name="es", bufs=D + 1)
            es8 = tc.alloc_tile_pool(name="es8", bufs=7)
            so = tc.alloc_tile_pool(name="so", bufs=4)
            rp = tc.alloc_tile_pool(name="rp", bufs=3)
            obp = tc.alloc_tile_pool(name="ob", bufs=4)
            ps1 = tc.alloc_tile_pool(name="ps1", bufs=1, space="PSUM")

            xc_t = [None] * 4

            def load_x(ch):
                # chunks >= 1 arrive fp8 for DoubleRow projections
                xc = xs.tile([128, NCT * 512], F8, name="xc", tag="xc")
                t0 = ch * 512
                nc.sync.dma_start(
                    out=xc.rearrange("p (ct t) -> p ct t", ct=NCT),
                    in_=xT8_d.rearrange("(ct p) t -> p ct t",
                                        p=128)[:, :, t0:t0 + 512])
                xc_t[ch] = xc

            # ---- head: one strided DMA per tensor (issue cost ~0.7us per
            # dma_start per queue; transfers pipeline behind the matmuls)
            xc0 = xs.tile([128, NCT * 512], BF16, name="xc", tag="xc")
            xc_t[0] = xc0
            nc.sync.dma_start(
                out=xc0.rearrange("p (ct t) -> p ct t", ct=NCT),
                in_=xT_d.rearrange("(ct p) t -> p ct t", p=128)[:, :, 0:512])
            nc.scalar.dma_start(
                out=Wq_sb.rearrange("p (ct c) -> p ct c", ct=NCT),
                in_=WqT_d.rearrange("(ct p) c -> p ct c", p=128))
            nc.gpsimd.dma_start(
                out=Wk_sb.rearrange("p (ct c) -> p ct c", ct=NCT),
                in_=WkT_d.rearrange("(ct p) c -> p ct c", p=128))
            nc.sync.dma_start(
                out=Wv_sb.rearrange("p (ct c) -> p ct c", ct=NCT),
                in_=WvT_d.rearrange("(ct p) c -> p ct c", p=128))
            nc.scalar.dma_start(out=bq_sb[:], in_=bq_d)
            nc.scalar.dma_start(out=bk_sb[:], in_=bk_d)
            nc.scalar.dma_start(out=bv_sb[:], in_=bv_d)
            nc.scalar.dma_start(out=pad_sb[:], in_=pad_d)
            nc.scalar.dma_start(out=tri_sb[:], in_=tri_d)
            nc.scalar.dma_start(out=idn_sb[:], in_=idn_d)
            nc.scalar.dma_start(out=trn_sb[:], in_=trn_d)
            nc.scalar.dma_start(
                out=Wq8_sb.rearrange("p (ct c) -> p ct c", ct=NCT),
                in_=WqT8_d.rearrange("(ct p) c -> p ct c", p=128))
            nc.gpsimd.dma_start(
                out=Wk8_sb.rearrange("p (ct c) -> p ct c", ct=NCT),
                in_=WkT8_d.rearrange("(ct p) c -> p ct c", p=128))
            nc.sync.dma_start(
                out=Wv8_sb.rearrange("p (ct c) -> p ct c", ct=NCT),
                in_=WvT8_d.rearrange("(ct p) c -> p ct c", p=128))
            for g in range(4):
                nc.gpsimd.dma_start(out=Wp_sb[:, g * C:(g + 1) * C],
                                    in_=WpT_d[g * 128:(g + 1) * 128, :])
            nc.gpsimd.partition_broadcast(bvb_sb[:], bv_sb[:])

            # ---- thunk-granular work units ------------------------------
            # Fill work is decomposed into ~1-matmul thunks so the pacing
            # can slot ~400ns of PE work between attention steps without
            # ever delaying the next QK matmul past the ACT exp window.
            def mk_proj_thunks(ch, kind, g):
                """QKV psum group as PE thunks; the last thunk drains.
                ch0 = bf16 (2 matmuls/thunk), else fp8 DoubleRow
                (1 matmul/thunk, two 128-contraction tiles each)."""
                st = {}
                fp8 = ch != 0
                t0 = ch * 512

                def drain():
                    pj = st["pj"]
                    if kind == "q":
                        nc.vector.tensor_scalar(
                            out=QT[:, g * T + t0: g * T + t0 + 512],
                            in0=pj[:], scalar1=SCALE,
                            scalar2=bq_sb[:, g:g + 1],
                            op0=ALU.mult, op1=ALU.add)
                    elif kind == "k":
                        nc.vector.tensor_scalar(
                            out=KT[:, g * T + t0: g * T + t0 + 512],
                            in0=pj[:], scalar1=bk_sb[:, g:g + 1],
                            scalar2=None, op0=ALU.add)
                    else:
                        kt = ch * 4 + g
                        nc.vector.tensor_tensor(
                            out=Vf8[:, kt // 2, kt % 2, :, 0:64],
                            in0=pj.rearrange("p (h c) -> p h c", h=8, c=64),
                            in1=bvb_sb.rearrange("p (h c) -> p h c", h=8, c=64),
                            op=ALU.add)
                        if kt < 4:
                            nc.vector.tensor_tensor(
                                out=Vf[:, kt, :, 0:64],
                                in0=pj.rearrange("p (h c) -> p h c", h=8, c=64),
                                in1=bvb_sb.rearrange("p (h c) -> p h c",
                                                     h=8, c=64),
                                op=ALU.add)

                def th_fp8(cp):
                    def run():
                        if cp == 0:
                            st["pj"] = ps1.tile([128, 512], F32, name="pj",
                                                tag="p1ps")
                        xcf = xc_t[ch].rearrange("p (ct t) -> p ct t", ct=NCT)
                        if kind == "q" or kind == "k":
                            W = (Wq8_sb if kind == "q" else Wk8_sb).rearrange(
                                "p (ct c) -> p ct c", ct=NCT)
                            nc.tensor.matmul(
                                out=st["pj"][:],
                                lhsT=W[:, 2 * cp:2 * cp + 2,
                                       g * 128:(g + 1) * 128],
                                rhs=xcf[:, 2 * cp:2 * cp + 2, :],
                                start=(cp == 0), stop=(cp == 3),
                                perf_mode=DR)
                        else:
                            Wvf = Wv8_sb.rearrange("p (ct c) -> p ct c",
                                                   ct=NCT)
                            nc.tensor.matmul(
                                out=st["pj"][:],
                                lhsT=xcf[:, 2 * cp:2 * cp + 2,
                                         g * 128:g * 128 + 128],
                                rhs=Wvf[:, 2 * cp:2 * cp + 2, :],
                                start=(cp == 0), stop=(cp == 3),
                                perf_mode=DR)
                        if cp == 3:
                            drain()
                    return run

                def th_bf16(ci):
                    def run():
                        if ci == 0:
                            if ch == 0:
                                # head: double-buffer via the (still idle)
                                # S-psum pool so units pipeline
                                big = pss.tile([128, 1024], F32, name="sAB",
                                               tag="sAB")
                                st["pj"] = big[:, 0:512]
                            else:
                                st["pj"] = ps1.tile([128, 512], F32,
                                                    name="pj", tag="p1ps")
                        for ct in (2 * ci, 2 * ci + 1):
                            if kind == "q" or kind == "k":
                                W = Wq_sb if kind == "q" else Wk_sb
                                nc.tensor.matmul(
                                    out=st["pj"][:],
                                    lhsT=W[:, ct * 512 + g * 128:
                                           ct * 512 + (g + 1) * 128],
                                    rhs=xc_t[ch][:, ct * 512:(ct + 1) * 512],
                                    start=(ct == 0), stop=(ct == NCT - 1))
                            else:
                                nc.tensor.matmul(
                                    out=st["pj"][:],
                                    lhsT=xc_t[ch][:, ct * 512 + g * 128:
                                                  ct * 512 + g * 128 + 128],
                                    rhs=Wv_sb[:, ct * 512:(ct + 1) * 512],
                                    start=(ct == 0), stop=(ct == NCT - 1))
                        if ci == 3:
                            drain()
                    return run

                return [th_fp8(c) if fp8 else th_bf16(c) for c in range(4)]

            def mk_out_thunks(ob_, qch):
                """Swapped output-projection group as 2 PE thunks: Wp is
                the stationary operand and Y^T streams as the MOVING
                operand, so the freshly-epilogue-written YT is never an
                LDWEIGHTS source (the PE hoists weight loads ahead of
                semaphore-protected matmuls). Output lands transposed
                [oc, q]; the host transposes back."""
                st = {}

                def th(half):
                    def run():
                        if half == 0:
                            st["po"] = ps1.tile([128, 512], F32, name="po",
                                                tag="p1ps")
                        for g in (2 * half, 2 * half + 1):
                            nc.tensor.matmul(
                                out=st["po"][:],
                                lhsT=Wp_sb[:, g * C + ob_ * 128:
                                           g * C + ob_ * 128 + 128],
                                rhs=YTg[g][:, qch * 512:(qch + 1) * 512],
                                start=(g == 0), stop=(g == 3))
                        if half == 1:
                            ob = obp.tile([128, 512], BF16, name="ob",
                                          tag="ob")
                            nc.vector.tensor_copy(ob[:], st["po"][:])
                            q_ = (nc.sync, nc.scalar, nc.gpsimd)[ob_ % 3]
                            q_.dma_start(
                                out=out_d[ob_ * 128:(ob_ + 1) * 128,
                                          qch * 512:(qch + 1) * 512],
                                in_=ob[:])
                    return run

                return [th(0), th(1)]

            def proj_chunk_thunks(ch):
                """All 12 proj units of a chunk. q/k g0 first (their moving
                reads of the x tile serialize the PE past the x DMA), then
                v as early as possible: V tiles are read as LDWEIGHTS by
                the AV matmuls, and the PE does not reliably honor write
                dependencies on freshly written weight sources unless they
                are many PE-queue instructions upstream."""
                th = []
                th += mk_proj_thunks(ch, "q", 0)
                th += mk_proj_thunks(ch, "k", 0)
                for ts in range(4):
                    th += mk_proj_thunks(ch, "v", ts)
                for g in (1, 2, 3):
                    th += mk_proj_thunks(ch, "q", g)
                    th += mk_proj_thunks(ch, "k", g)
                return th

            # ---- head: proj(0) for g0/g1 and all v up front; g2/g3 ride
            # as round-0 fill (their QK consumers are 6-8 slots downstream,
            # far enough for the LDWEIGHTS distance rule)
            for th in mk_proj_thunks(0, "q", 0) + mk_proj_thunks(0, "k", 0):
                th()
            for ts in range(4):
                for th in mk_proj_thunks(0, "v", ts):
                    th()
            for th in mk_proj_thunks(0, "q", 1) + mk_proj_thunks(0, "k", 1):
                th()

            # ---- flat event stream across all rounds --------------------
            # QK/exp events for every (qc, g, kt); the AV for event j fires
            # at slot j+D, so g-group and round boundaries overlap and ACT
            # never waits out an AV drain tail.
            EV = []
            for qc in range(4):
                for g in range(4):
                    for kt in range(4 * qc + 4):
                        EV.append((qc, g, kt))
            NEV = len(EV)

            e_l = {}     # (qc, g, kt) -> e tile
            off_l = {}   # (qc, g, kt) -> toff
            o_l = {}     # (qc, g) -> oAB psum
            fillq = []
            need = {}    # (qc, g, kt) -> fillq mark that must be emitted first
            pace = {"acc": 0.0, "rate": 0.0, "fi": 0}

            def pump():
                pace["acc"] += pace["rate"]
                while pace["acc"] >= 1.0 and pace["fi"] < len(fillq):
                    fillq[pace["fi"]]()
                    pace["fi"] += 1
                    pace["acc"] -= 1.0

            def force(mark):
                # producers must be EMITTED before their consumer events, or
                # the dependency tracker never sees the ordering (race)
                while pace["fi"] < mark:
                    fillq[pace["fi"]]()
                    pace["fi"] += 1

            def epilogue(qc, g):
                # normalize by rowsum (row 64 of each half), write Y^T.
                # The rowsum hop through rs_ is a DVE copy: only the copy
                # uop supports the partition-64 -> partition-0 move.
                q0 = qc * 512
                oA, oB = o_l.pop((qc, g))
                rs_ = rp.tile([1, 1024], F32, name="rs_", tag="rs_")
                rr_ = rp.tile([1, 1024], F32, name="rr_", tag="rr_")
                rbA = rp.tile([64, 512], F32, name="rbA", tag="rbA")
                rbB = rp.tile([64, 512], F32, name="rbB", tag="rbB")
                nc.vector.tensor_copy(rs_[:, 0:512], oA[64:65, :])
                nc.vector.reciprocal_approx_fast(rr_[:, 0:512], rs_[:, 0:512])
                nc.gpsimd.partition_broadcast(rbA[:], rr_[:, 0:512])
                nc.vector.tensor_copy(rs_[:, 512:1024], oB[64:65, :])
                nc.vector.tensor_mul(
                    YTg[g][0:64, q0: q0 + 512], oA[0:64, :], rbA[:])
                nc.vector.reciprocal_approx_fast(rr_[:, 512:1024],
                                                 rs_[:, 512:1024])
                nc.gpsimd.partition_broadcast(rbB[:], rr_[:, 512:1024])
                nc.vector.tensor_mul(
                    YTg[g][64:128, q0: q0 + 512], oB[0:64, :], rbB[:])

            for i in range(NEV + D):
                if i < NEV:
                    qc, g, kt = EV[i]
                    if g == 0 and kt == 0:
                        # round head: emit everything this round's attention
                        # reads, then enqueue the next round's fill
                        force(need.get((qc, 0, 0), 0))
                        if qc == 0:
                            for gg in (2, 3):
                                fillq.extend(mk_proj_thunks(0, "q", gg))
                                fillq.extend(mk_proj_thunks(0, "k", gg))
                                need[(0, gg, 0)] = len(fillq)
                            load_x(1)
                            fillq.extend(proj_chunk_thunks(1))
                            need[(1, 0, 0)] = len(fillq)
                        elif qc < 3:
                            load_x(qc + 1)
                            fillq.extend(proj_chunk_thunks(qc + 1))
                            need[(qc + 1, 0, 0)] = len(fillq)
                        else:
                            # round 3 is the ACT-rich round: park all the
                            # output-projection fill here
                            for ob_ in range(8):
                                fillq.extend(mk_out_thunks(ob_, 0))
                            for ob_ in range(8):
                                fillq.extend(mk_out_thunks(ob_, 1))
                        wlen = (4 * qc + 4) * 4
                        nadd = len(fillq) - pace["fi"]
                        pace["rate"] = 1.0 * nadd / wlen if qc else 5.0
                    elif kt == 0:
                        force(need.get((qc, g, 0), 0))
                    if qc == 3 and g == 1 and kt == 0:
                        # qch2 output columns read YTg written by round-2
                        # epilogues; the last lands D-1 slots into round 3,
                        # so enqueue these well after it
                        for ob_ in range(8):
                            fillq.extend(mk_out_thunks(ob_, 2))
                    q0 = qc * 512
                    gq = g * T
                    k0 = kt * 128
                    toff = 128 * (kt - 4 * qc) if kt >= 4 * qc else 0
                    off_l[(qc, g, kt)] = toff
                    diag = kt >= 4 * qc
                    sAB = pss.tile([128, 1024], F32, name="sAB", tag="sAB")
                    nc.tensor.matmul(
                        out=sAB[:, toff:512],
                        lhsT=KT[0:64, gq + k0: gq + k0 + 128],
                        rhs=QT[0:64, gq + q0 + toff: gq + q0 + 512],
                        start=True, stop=True,
                    )
                    nc.tensor.matmul(
                        out=sAB[:, 512 + toff:1024],
                        lhsT=KT[64:128, gq + k0: gq + k0 + 128],
                        rhs=QT[64:128, gq + q0 + toff: gq + q0 + 512],
                        start=True, stop=True, tile_position=(64, 0),
                    )
                    s3 = sAB.rearrange("p (h w) -> p h w", h=2, w=512)
                    trnf = trn_sb.rearrange("p (h w) -> p h w", h=2, w=256)
                    if diag:
                        # causal masking on the PE: add a -1e30 strict lower
                        # triangle (identity stationary, NEG-tri moving) into
                        # the S psum of both head halves before exp
                        nc.tensor.matmul(
                            out=s3[:, :, toff:toff + 128],
                            lhsT=idn_sb[:],
                            rhs=trnf[:, :, 128:256],
                            start=False, stop=True, skip_group_check=True,
                        )
                    if qc == 0:
                        eAB = es.tile([128, 1024], BF16, name="eAB", tag="eAB")
                        e3 = eAB.rearrange("p (h w) -> p h w", h=2, w=512)
                        nc.scalar.activation(
                            e3[:, :, toff:512], s3[:, :, toff:512],
                            AF.Exp, bias=pad_sb[:, kt:kt + 1])
                        e_l[(qc, g, kt)] = eAB
                    else:
                        # fp8 pair tile: slot j = kt parity
                        j = kt % 2
                        if j == 0:
                            e2 = es8.tile([128, 2048], F8, name="e2", tag="e2")
                        else:
                            e2 = e_l[(qc, g, kt - 1)]
                        e_l[(qc, g, kt)] = e2
                        e4 = e2.rearrange("p (j h w) -> p j h w",
                                          j=2, h=2, w=512)
                        elo = toff
                        if diag and j == 1:
                            # odd member: overwrite the 128-col causal gap
                            # with -1e30 so exp writes exact 0s there
                            nc.tensor.matmul(
                                out=s3[:, :, toff - 128:toff],
                                lhsT=idn_sb[:],
                                rhs=trnf[:, :, 0:128],
                                start=True, stop=True, skip_group_check=True,
                            )
                            elo = toff - 128
                        nc.scalar.activation(
                            e4[:, j, :, elo:512], s3[:, :, elo:512],
                            AF.Exp, bias=pad_sb[:, kt:kt + 1])
                jj = i - D
                if 0 <= jj < NEV:
                    qc, g, pv = EV[jj]
                    kmax = 4 * qc + 4
                    gq = g * T
                    if pv == 0:
                        o_l[(qc, g)] = (
                            pso.tile([65, 512], F32, name="oA", tag="o"),
                            pso.tile([65, 512], F32, name="oB", tag="o"))
                    oA, oB = o_l[(qc, g)]
                    if qc == 0:
                        toff = off_l[(qc, g, pv)]
                        vbase = pv * 520
                        eAB = e_l.pop((qc, g, pv))
                        nc.tensor.matmul(
                            out=oA[:, toff:512],
                            lhsT=Vt[:, vbase + 130 * g: vbase + 130 * g + 65],
                            rhs=eAB[:, toff:512],
                            start=(pv == 0), stop=(pv == kmax - 1),
                        )
                        nc.tensor.matmul(
                            out=oB[:, toff:512],
                            lhsT=Vt[:, vbase + 130 * g + 65:
                                    vbase + 130 * g + 130],
                            rhs=eAB[:, 512 + toff:1024],
                            start=(pv == 0), stop=(pv == kmax - 1),
                        )
                    elif pv % 2 == 1:
                        # DoubleRow AV over the k-tile pair (pv-1, pv)
                        pr = pv // 2
                        toff = off_l[(qc, g, pv - 1)]
                        e4 = e_l.pop((qc, g, pv)).rearrange(
                            "p (j h w) -> p j h w", j=2, h=2, w=512)
                        e_l.pop((qc, g, pv - 1))
                        npr = kmax // 2
                        nc.tensor.matmul(
                            out=oA[:, toff:512],
                            lhsT=Vf8[:, pr, :, 2 * g, 0:65],
                            rhs=e4[:, :, 0, toff:512],
                            start=(pr == 0), stop=(pr == npr - 1),
                            perf_mode=DR,
                        )
                        nc.tensor.matmul(
                            out=oB[:, toff:512],
                            lhsT=Vf8[:, pr, :, 2 * g + 1, 0:65],
                            rhs=e4[:, :, 1, toff:512],
                            start=(pr == 0), stop=(pr == npr - 1),
                            perf_mode=DR,
                        )
                    if pv == kmax - 1:
                        epilogue(qc, g)
                pump()
            # drain any fill left over
            while pace["fi"] < len(fillq):
                fillq[pace["fi"]]()
                pace["fi"] += 1

            # ---- tail: last chunk of the output projection ----
            # release the attention pools first so the tail gets a deep
            # psum pool: six units can then hold open accumulation groups
            # and their g0-g2 matmuls cover the final epilogue's latency
            ps1.release()
            obp.release()
            rp.release()
            so.release()
            es8.release()
            es.release()
            pso.release()
            pss.release()
            ptail = tc.alloc_tile_pool(name="ptail", bufs=8, space="PSUM")
            obt = tc.alloc_tile_pool(name="obt", bufs=4)
            pos = []
            for ob_ in range(8):
                po = ptail.tile([128, 512], F32, name="po2", tag="po2")
                pos.append(po)
                for g in range(3):
                    nc.tensor.matmul(
                        out=po[:],
                        lhsT=Wp_sb[:, g * C + ob_ * 128:
                                   g * C + ob_ * 128 + 128],
                        rhs=YTg[g][:, 3 * 512:4 * 512],
                        start=(g == 0), stop=False,
                    )
            for ob_ in range(8):
                po = pos[ob_]
                nc.tensor.matmul(
                    out=po[:],
                    lhsT=Wp_sb[:, 3 * C + ob_ * 128: 3 * C + ob_ * 128 + 128],
                    rhs=YTg[3][:, 3 * 512:4 * 512],
                    start=False, stop=True,
                )
                ob = obt.tile([128, 512], BF16, name="ob2", tag="ob2")
                nc.vector.tensor_copy(ob[:], po[:])
                q_ = (nc.sync, nc.scalar, nc.gpsimd)[ob_ % 3]
                q_.dma_start(
                    out=out_d[ob_ * 128:(ob_ + 1) * 128, 3 * 512:4 * 512],
                    in_=ob[:])
            obt.release()
            ptail.release()
            xs.release()

    nc.compile()
    return nc


def _in_maps(x, Wk, bk, Wq, bq, Wv, bv, Wp, bp, padding_mask):
    maps = []
    bf16 = ml_dtypes.bfloat16
    rows = np.arange(128)[:, None]
    cols = np.arange(128)[None, :]
    tri01 = (cols >= rows).astype(np.float32)
    trib = np.where(cols < rows, NEG, 0.0).astype(np.float32)
    blk = np.concatenate([np.full((128, 128), NEG, np.float32), trib], axis=1)
    trneg = np.concatenate([blk, blk], axis=1)
    f8 = ml_dtypes.float8_e4m3fn
    for core in range(8):
        b, half = divmod(core, 2)
        hs = slice(half * IC, (half + 1) * IC)
        xTb = np.ascontiguousarray(x[b].T)
        maps.append({
            "xT": xTb.astype(bf16),
            "xT8": xTb.astype(f8),
            "WqT8": np.ascontiguousarray(Wq[hs, :].T).astype(f8),
            "WkT8": np.ascontiguousarray(Wk[hs, :].T).astype(f8),
            "WvT8": np.ascontiguousarray(Wv[hs, :].T).astype(f8),
            "WqT": np.ascontiguousarray(Wq[hs, :].T).astype(bf16),
            "WkT": np.ascontiguousarray(Wk[hs, :].T).astype(bf16),
            "WvT": np.ascontiguousarray(Wv[hs, :].T).astype(bf16),
            "WpT": np.ascontiguousarray(Wp[:, hs].T).astype(bf16),
            "bqs": np.ascontiguousarray((bq[hs] * SCALE).reshape(4, 128).T),
            "bks": np.ascontiguousarray(bk[hs].reshape(4, 128).T),
            "bvr": bv[hs].reshape(1, IC).copy(),
            "padb": np.ascontiguousarray(
                np.where(padding_mask[b] != 0, 0.0, NEG)
                .astype(np.float32).reshape(NKT, 128).T),
            "tri01": tri01.astype(bf16),
            "idn": np.eye(128, dtype=np.float32).astype(bf16),
            "trneg": trneg.astype(bf16),
            "ones8": np.ones((128, 8), bf16),
        })
    return maps


def _run(inputs, trace=False, **kw):
    if "nc" not in _CACHE:
        _CACHE["nc"] = _build()
    nc = _CACHE["nc"]
    ins = {k: np.asarray(v, dtype=np.float32) if k != "padding_mask"
           else np.asarray(v) for k, v in inputs.items()}
    maps = _in_maps(**ins)
    res = run_bass_kernel_spmd(nc, maps, core_ids=list(range(8)), trace=trace, **kw)
    bp = np.asarray(inputs["bp"], np.float32)
    y = np.empty((B, T, C), np.float32)
    for b in range(B):
        y[b] = (res.results[2 * b]["out"].astype(np.float32)
                + res.results[2 * b + 1]["out"].astype(np.float32)).T + bp
    return y, res


def kernel(**inputs):
    y, _ = _run(inputs, trace=False)
    return y



# revision 38
# speedup vs baseline: 1.0121x; 1.0121x over previous
"""Causal self-attention TRN2 Bass kernel (phase-interleaved).

Problem: B=4, T=2048, C=1024, H=16 heads (HD=64), torch-Linear semantics
(y = x @ W.T + b), causal + padding mask, softmax, output projection.

Sharding: 8 cores = (batch b in 0..3) x (head-half in 0..1). Each core
handles one batch and 8 heads (512 of the 1024 channels of QKV / of the
contraction dim of the output projection). The two half-cores of a batch
produce partial output projections that the host sums (plus bp).

Design (376us baseline -> ~276us):
  - Attention q-chunks run ASCENDING; PE-bound projection and
    output-projection units are interleaved as filler between the
    ACT-bound softmax steps so the Tensor engine never idles while ACT
    grinds exp(). Fill is placed by measured engine balance: rounds 1-2
    are PE-bound (proj fill only); round 3 is ACT-bound with ~17us of
    PE slack, so ALL output-projection fill (chunks 1 and 2) parks
    there:
      proj(0) -> [attn(0)|proj(1)] -> [attn(1)|proj(2)|out(0)]
              -> [attn(2)|proj(3)] -> [attn(3)|out(1)|out(2)] -> out(3)
  - All inputs shipped bf16 (same matmul rate, half the DMA bytes);
    V and exp(S) tiles bf16 (faster LDWEIGHTS, 2x DVE mask multiply).
  - DMA issue spread across the 3 capable queues (sync/scalar/gpsimd),
    ~700ns per dma_start per queue; first x/Wq tiles split in half so
    the first matmul starts ~10us in.
  - Causal diag mask: DVE multiply with a 0/1 triangle after exp
    (no PE identity-matmuls); V bias added during the DVE psum drain.
  - Rowsum via a ones-column in the V stationary ([V|1]); normalization:
    psum row -> SBUF copy -> reciprocal_approx_fast -> partition
    broadcast -> DVE multiply (approx-recip cannot read PSUM directly).
  - Y^T kept as four per-head-group tiles so output-projection reads
    depend only on the epilogue that wrote them; the last four token
    tiles use a 6-buffer psum pool (attention pools released first) so
    their partial accumulations cover the final epilogue's latency.
  - PSUM: 2 proj/out + 2x2 S + 1x2 O banks = 8 during attention.
  - Engine budget: PE ~245us (the roofline), ACT ~158us of exp,
    DVE ~130us of drains/masks/normalize, sync ~45us of DMA issue.
"""

import ml_dtypes
import numpy as np

import concourse.mybir as mybir
import concourse.tile as tile
from concourse import bacc
from concourse.bass_utils import run_bass_kernel_spmd

F32 = mybir.dt.float32
F32R = mybir.dt.float32r
BF16 = mybir.dt.bfloat16
F8 = mybir.dt.float8e4
DR = mybir.MatmulPerfMode.DoubleRow
AF = mybir.ActivationFunctionType
ALU = mybir.AluOpType

B, T, C, H = 4, 2048, 1024, 16
HD = C // H          # 64
IC = C // 2          # 512 channels per core (8 heads)
NKT = T // 128       # 16 k-tiles
NCT = C // 128       # 8 contraction tiles for QKV
NEG = -1.0e30
SCALE = 1.0 / np.sqrt(HD)
D = 8                # S->AV pipeline lag (in k-tile steps)

_CACHE = {}


def _build():
    nc = bacc.Bacc("TRN2", target_bir_lowering=False, debug=False)

    xT_d = nc.dram_tensor("xT", [C, T], BF16, kind="ExternalInput").ap()
    xT8_d = nc.dram_tensor("xT8", [C, T], F8, kind="ExternalInput").ap()
    WqT8_d = nc.dram_tensor("WqT8", [C, IC], F8, kind="ExternalInput").ap()
    WkT8_d = nc.dram_tensor("WkT8", [C, IC], F8, kind="ExternalInput").ap()
    WvT8_d = nc.dram_tensor("WvT8", [C, IC], F8, kind="ExternalInput").ap()
    WqT_d = nc.dram_tensor("WqT", [C, IC], BF16, kind="ExternalInput").ap()
    WkT_d = nc.dram_tensor("WkT", [C, IC], BF16, kind="ExternalInput").ap()
    WvT_d = nc.dram_tensor("WvT", [C, IC], BF16, kind="ExternalInput").ap()
    WpT_d = nc.dram_tensor("WpT", [IC, C], BF16, kind="ExternalInput").ap()
    bq_d = nc.dram_tensor("bqs", [128, 4], F32, kind="ExternalInput").ap()
    bk_d = nc.dram_tensor("bks", [128, 4], F32, kind="ExternalInput").ap()
    bv_d = nc.dram_tensor("bvr", [1, IC], F32, kind="ExternalInput").ap()
    pad_d = nc.dram_tensor("padb", [128, NKT], F32, kind="ExternalInput").ap()
    tri_d = nc.dram_tensor("tri01", [128, 128], BF16, kind="ExternalInput").ap()
    idn_d = nc.dram_tensor("idn", [128, 128], BF16, kind="ExternalInput").ap()
    trn_d = nc.dram_tensor("trneg", [128, 512], BF16, kind="ExternalInput").ap()
    ones_d = nc.dram_tensor("ones8", [128, 8], BF16, kind="ExternalInput").ap()
    out_d = nc.dram_tensor("out", [C, T], BF16, kind="ExternalOutput").ap()

    with tile.TileContext(nc) as tc:
        with tc.tile_pool(name="pp", bufs=1) as pp:
            QT = pp.tile([128, 4 * T], BF16, name="QT")
            KT = pp.tile([128, 4 * T], BF16, name="KT")
            Vt = pp.tile([128, 4 * 520], BF16, name="Vt")
            Vt8 = pp.tile([128, NKT * 528], F8, name="Vt8")
            YTg = [pp.tile([128, T], BF16, name=f"YT{i}", uniquify=False)
                   for i in range(4)]
            Wp_sb = pp.tile([128, 4 * C], BF16, name="Wp_sb")
            Wq_sb = pp.tile([128, NCT * 512], BF16, name="Wq_sb")
            Wk_sb = pp.tile([128, NCT * 512], BF16, name="Wk_sb")
            Wv_sb = pp.tile([128, NCT * 512], BF16, name="Wv_sb")
            Wq8_sb = pp.tile([128, NCT * 512], F8, name="Wq8_sb")
            Wk8_sb = pp.tile([128, NCT * 512], F8, name="Wk8_sb")
            Wv8_sb = pp.tile([128, NCT * 512], F8, name="Wv8_sb")
            bq_sb = pp.tile([128, 4], F32, name="bq_sb")
            bk_sb = pp.tile([128, 4], F32, name="bk_sb")
            bv_sb = pp.tile([1, IC], F32, name="bv_sb")
            bvb_sb = pp.tile([128, IC], F32, name="bvb_sb")
            pad_sb = pp.tile([128, NKT], F32, name="pad_sb")
            tri_sb = pp.tile([128, 128], BF16, name="tri_sb")
            idn_sb = pp.tile([128, 128], BF16, name="idn_sb")
            trn_sb = pp.tile([128, 512], BF16, name="trn_sb")
            one8_sb = pp.tile([128, 8], BF16, name="one8_sb")

            # bf16 V for the first 4 k-tiles (qc=0); fp8 k-tile-paired V
            # (pair pr holds k-tiles 2pr, 2pr+1) for the DoubleRow AV.
            Vf = Vt.rearrange("p (k h c) -> p k h c", k=4, h=8, c=65)
            # c padded to 66 so the pair-dim (j) byte stride is 528,
            # a multiple of 16 as the DoubleRow LDWEIGHTS ISA requires
            Vf8 = Vt8.rearrange("p (pr j h c) -> p pr j h c",
                                pr=NKT // 2, j=2, h=8, c=66)
            nc.gpsimd.dma_start(out=one8_sb[:], in_=ones_d)
            for kt in range(4):
                nc.vector.tensor_copy(Vf[:, kt, :, 64], one8_sb[:, :, None])
            for pr in range(NKT // 2):
                for j in range(2):
                    nc.vector.tensor_copy(Vf8[:, pr, j, :, 64],
                                          one8_sb[:, :, None])

            xs = tc.alloc_tile_pool(name="xs", bufs=2)
            pss = tc.alloc_tile_pool(name="pss", bufs=2, space="PSUM")
            pso = tc.alloc_tile_pool(name="pso", bufs=3, space="PSUM")
            es = tc.alloc_tile_pool(name="es", bufs=D + 1)
            es8 = tc.alloc_tile_pool(name="es8", bufs=6)
            so = tc.alloc_tile_pool(name="so", bufs=4)
            rp = tc.alloc_tile_pool(name="rp", bufs=2)
            obp = tc.alloc_tile_pool(name="ob", bufs=3)
            ps1 = tc.alloc_tile_pool(name="ps1", bufs=1, space="PSUM")

            xc_t = [None] * 4

            def load_x(ch):
                # chunks >= 1 arrive fp8 for DoubleRow projections
                xc = xs.tile([128, NCT * 512], F8, name="xc", tag="xc")
                t0 = ch * 512
                nc.sync.dma_start(
                    out=xc.rearrange("p (ct t) -> p ct t", ct=NCT),
                    in_=xT8_d.rearrange("(ct p) t -> p ct t",
                                        p=128)[:, :, t0:t0 + 512])
                xc_t[ch] = xc

            # ---- head: one strided DMA per tensor (issue cost ~0.7us per
            # dma_start per queue; transfers pipeline behind the matmuls)
            xc0 = xs.tile([128, NCT * 512], BF16, name="xc", tag="xc")
            xc_t[0] = xc0
            nc.sync.dma_start(
                out=xc0.rearrange("p (ct t) -> p ct t", ct=NCT),
                in_=xT_d.rearrange("(ct p) t -> p ct t", p=128)[:, :, 0:512])
            nc.scalar.dma_start(
                out=Wq_sb.rearrange("p (ct c) -> p ct c", ct=NCT),
                in_=WqT_d.rearrange("(ct p) c -> p ct c", p=128))
            nc.gpsimd.dma_start(
                out=Wk_sb.rearrange("p (ct c) -> p ct c", ct=NCT),
                in_=WkT_d.rearrange("(ct p) c -> p ct c", p=128))
            nc.sync.dma_start(
                out=Wv_sb.rearrange("p (ct c) -> p ct c", ct=NCT),
                in_=WvT_d.rearrange("(ct p) c -> p ct c", p=128))
            nc.scalar.dma_start(out=bq_sb[:], in_=bq_d)
            nc.scalar.dma_start(out=bk_sb[:], in_=bk_d)
            nc.scalar.dma_start(out=bv_sb[:], in_=bv_d)
            nc.scalar.dma_start(out=pad_sb[:], in_=pad_d)
            nc.scalar.dma_start(out=tri_sb[:], in_=tri_d)
            nc.scalar.dma_start(out=idn_sb[:], in_=idn_d)
            nc.scalar.dma_start(out=trn_sb[:], in_=trn_d)
            nc.scalar.dma_start(
                out=Wq8_sb.rearrange("p (ct c) -> p ct c", ct=NCT),
                in_=WqT8_d.rearrange("(ct p) c -> p ct c", p=128))
            nc.gpsimd.dma_start(
                out=Wk8_sb.rearrange("p (ct c) -> p ct c", ct=NCT),
                in_=WkT8_d.rearrange("(ct p) c -> p ct c", p=128))
            nc.sync.dma_start(
                out=Wv8_sb.rearrange("p (ct c) -> p ct c", ct=NCT),
                in_=WvT8_d.rearrange("(ct p) c -> p ct c", p=128))
            for g in range(4):
                nc.gpsimd.dma_start(out=Wp_sb[:, g * C:(g + 1) * C],
                                    in_=WpT_d[g * 128:(g + 1) * 128, :])
            nc.gpsimd.partition_broadcast(bvb_sb[:], bv_sb[:])

            # ---- thunk-granular work units ------------------------------
            # Fill work is decomposed into ~1-matmul thunks so the pacing
            # can slot ~400ns of PE work between attention steps without
            # ever delaying the next QK matmul past the ACT exp window.
            def mk_proj_thunks(ch, kind, g):
                """QKV psum group as PE thunks; the last thunk drains.
                ch0 = bf16 (2 matmuls/thunk), else fp8 DoubleRow
                (1 matmul/thunk, two 128-contraction tiles each)."""
                st = {}
                fp8 = ch != 0
                t0 = ch * 512

                def drain():
                    pj = st["pj"]
                    if kind == "q":
                        nc.vector.tensor_scalar(
                            out=QT[:, g * T + t0: g * T + t0 + 512],
                            in0=pj[:], scalar1=SCALE,
                            scalar2=bq_sb[:, g:g + 1],
                            op0=ALU.mult, op1=ALU.add)
                    elif kind == "k":
                        nc.vector.tensor_scalar(
                            out=KT[:, g * T + t0: g * T + t0 + 512],
                            in0=pj[:], scalar1=bk_sb[:, g:g + 1],
                            scalar2=None, op0=ALU.add)
                    else:
                        kt = ch * 4 + g
                        nc.vector.tensor_tensor(
                            out=Vf8[:, kt // 2, kt % 2, :, 0:64],
                            in0=pj.rearrange("p (h c) -> p h c", h=8, c=64),
                            in1=bvb_sb.rearrange("p (h c) -> p h c", h=8, c=64),
                            op=ALU.add)
                        if kt < 4:
                            nc.vector.tensor_tensor(
                                out=Vf[:, kt, :, 0:64],
                                in0=pj.rearrange("p (h c) -> p h c", h=8, c=64),
                                in1=bvb_sb.rearrange("p (h c) -> p h c",
                                                     h=8, c=64),
                                op=ALU.add)

                def th_fp8(cp):
                    def run():
                        if cp == 0:
                            st["pj"] = ps1.tile([128, 512], F32, name="pj",
                                                tag="p1ps")
                        xcf = xc_t[ch].rearrange("p (ct t) -> p ct t", ct=NCT)
                        if kind == "q" or kind == "k":
                            W = (Wq8_sb if kind == "q" else Wk8_sb).rearrange(
                                "p (ct c) -> p ct c", ct=NCT)
                            nc.tensor.matmul(
                                out=st["pj"][:],
                                lhsT=W[:, 2 * cp:2 * cp + 2,
                                       g * 128:(g + 1) * 128],
                                rhs=xcf[:, 2 * cp:2 * cp + 2, :],
                                start=(cp == 0), stop=(cp == 3),
                                perf_mode=DR)
                        else:
                            Wvf = Wv8_sb.rearrange("p (ct c) -> p ct c",
                                                   ct=NCT)
                            nc.tensor.matmul(
                                out=st["pj"][:],
                                lhsT=xcf[:, 2 * cp:2 * cp + 2,
                                         g * 128:g * 128 + 128],
                                rhs=Wvf[:, 2 * cp:2 * cp + 2, :],
                                start=(cp == 0), stop=(cp == 3),
                                perf_mode=DR)
                        if cp == 3:
                            drain()
                    return run

                def th_bf16(ci):
                    def run():
                        if ci == 0:
                            if ch == 0:
                                # head: double-buffer via the (still idle)
                                # S-psum pool so units pipeline
                                big = pss.tile([128, 1024], F32, name="sAB",
                                               tag="sAB")
                                st["pj"] = big[:, 0:512]
                            else:
                                st["pj"] = ps1.tile([128, 512], F32,
                                                    name="pj", tag="p1ps")
                        for ct in (2 * ci, 2 * ci + 1):
                            if kind == "q" or kind == "k":
                                W = Wq_sb if kind == "q" else Wk_sb
                                nc.tensor.matmul(
                                    out=st["pj"][:],
                                    lhsT=W[:, ct * 512 + g * 128:
                                           ct * 512 + (g + 1) * 128],
                                    rhs=xc_t[ch][:, ct * 512:(ct + 1) * 512],
                                    start=(ct == 0), stop=(ct == NCT - 1))
                            else:
                                nc.tensor.matmul(
                                    out=st["pj"][:],
                                    lhsT=xc_t[ch][:, ct * 512 + g * 128:
                                                  ct * 512 + g * 128 + 128],
                                    rhs=Wv_sb[:, ct * 512:(ct + 1) * 512],
                                    start=(ct == 0), stop=(ct == NCT - 1))
                        if ci == 3:
                            drain()
                    return run

                return [th_fp8(c) if fp8 else th_bf16(c) for c in range(4)]

            def mk_out_thunks(ob_, qch):
                """Swapped output-projection group as 2 PE thunks: Wp is
                the stationary operand and Y^T streams as the MOVING
                operand, so the freshly-epilogue-written YT is never an
                LDWEIGHTS source (the PE hoists weight loads ahead of
                semaphore-protected matmuls). Output lands transposed
                [oc, q]; the host transposes back."""
                st = {}

                def th(half):
                    def run():
                        if half == 0:
                            st["po"] = ps1.tile([128, 512], F32, name="po",
                                                tag="p1ps")
                        for g in (2 * half, 2 * half + 1):
                            nc.tensor.matmul(
                                out=st["po"][:],
                                lhsT=Wp_sb[:, g * C + ob_ * 128:
                                           g * C + ob_ * 128 + 128],
                                rhs=YTg[g][:, qch * 512:(qch + 1) * 512],
                                start=(g == 0), stop=(g == 3))
                        if half == 1:
                            ob = obp.tile([128, 512], BF16, name="ob",
                                          tag="ob")
                            nc.vector.tensor_copy(ob[:], st["po"][:])
                            q_ = (nc.sync, nc.scalar, nc.gpsimd)[ob_ % 3]
                            q_.dma_start(
                                out=out_d[ob_ * 128:(ob_ + 1) * 128,
                                          qch * 512:(qch + 1) * 512],
                                in_=ob[:])
                    return run

                return [th(0), th(1)]

            def proj_chunk_thunks(ch):
                """All 12 proj units of a chunk. q/k g0 first (their moving
                reads of the x tile serialize the PE past the x DMA), then
                v as early as possible: V tiles are read as LDWEIGHTS by
                the AV matmuls, and the PE does not reliably honor write
                dependencies on freshly written weight sources unless they
                are many PE-queue instructions upstream."""
                th = []
                th += mk_proj_thunks(ch, "q", 0)
                th += mk_proj_thunks(ch, "k", 0)
                for ts in range(4):
                    th += mk_proj_thunks(ch, "v", ts)
                for g in (1, 2, 3):
                    th += mk_proj_thunks(ch, "q", g)
                    th += mk_proj_thunks(ch, "k", g)
                return th

            # ---- head: proj(0) for g0/g1 and all v up front; g2/g3 ride
            # as round-0 fill (their QK consumers are 6-8 slots downstream,
            # far enough for the LDWEIGHTS distance rule)
            for th in mk_proj_thunks(0, "q", 0) + mk_proj_thunks(0, "k", 0):
                th()
            for ts in range(4):
                for th in mk_proj_thunks(0, "v", ts):
                    th()
            for th in mk_proj_thunks(0, "q", 1) + mk_proj_thunks(0, "k", 1):
                th()

            # ---- flat event stream across all rounds --------------------
            # QK/exp events for every (qc, g, kt); the AV for event j fires
            # at slot j+D, so g-group and round boundaries overlap and ACT
            # never waits out an AV drain tail.
            EV = []
            for qc in range(4):
                for g in range(4):
                    for kt in range(4 * qc + 4):
                        EV.append((qc, g, kt))
            NEV = len(EV)

            e_l = {}     # (qc, g, kt) -> e tile
            off_l = {}   # (qc, g, kt) -> toff
            o_l = {}     # (qc, g) -> oAB psum
            fillq = []
            need = {}    # (qc, g, kt) -> fillq mark that must be emitted first
            pace = {"acc": 0.0, "rate": 0.0, "fi": 0}

            def pump():
                pace["acc"] += pace["rate"]
                while pace["acc"] >= 1.0 and pace["fi"] < len(fillq):
                    fillq[pace["fi"]]()
                    pace["fi"] += 1
                    pace["acc"] -= 1.0

            def force(mark):
                # producers must be EMITTED before their consumer events, or
                # the dependency tracker never sees the ordering (race)
                while pace["fi"] < mark:
                    fillq[pace["fi"]]()
                    pace["fi"] += 1

            def epilogue(qc, g):
                # normalize by rowsum (row 64 of each half), write Y^T.
                # The rowsum hop through rs_ is a DVE copy: only the copy
                # uop supports the partition-64 -> partition-0 move.
                q0 = qc * 512
                oA, oB = o_l.pop((qc, g))
                rs_ = rp.tile([1, 1024], F32, name="rs_", tag="rs_")
                rr_ = rp.tile([1, 1024], F32, name="rr_", tag="rr_")
                rbA = rp.tile([64, 512], F32, name="rbA", tag="rbA")
                rbB = rp.tile([64, 512], F32, name="rbB", tag="rbB")
                nc.vector.tensor_copy(rs_[:, 0:512], oA[64:65, :])
                nc.vector.reciprocal_approx_fast(rr_[:, 0:512], rs_[:, 0:512])
                nc.gpsimd.partition_broadcast(rbA[:], rr_[:, 0:512])
                nc.vector.tensor_copy(rs_[:, 512:1024], oB[64:65, :])
                nc.vector.tensor_mul(
                    YTg[g][0:64, q0: q0 + 512], oA[0:64, :], rbA[:])
                nc.vector.reciprocal_approx_fast(rr_[:, 512:1024],
                                                 rs_[:, 512:1024])
                nc.gpsimd.partition_broadcast(rbB[:], rr_[:, 512:1024])
                nc.vector.tensor_mul(
                    YTg[g][64:128, q0: q0 + 512], oB[0:64, :], rbB[:])

            for i in range(NEV + D):
                if i < NEV:
                    qc, g, kt = EV[i]
                    if g == 0 and kt == 0:
                        # round head: emit everything this round's attention
                        # reads, then enqueue the next round's fill
                        force(need.get((qc, 0, 0), 0))
                        if qc == 0:
                            for gg in (2, 3):
                                fillq.extend(mk_proj_thunks(0, "q", gg))
                                fillq.extend(mk_proj_thunks(0, "k", gg))
                                need[(0, gg, 0)] = len(fillq)
                            load_x(1)
                            fillq.extend(proj_chunk_thunks(1))
                            need[(1, 0, 0)] = len(fillq)
                        elif qc < 3:
                            load_x(qc + 1)
                            fillq.extend(proj_chunk_thunks(qc + 1))
                            need[(qc + 1, 0, 0)] = len(fillq)
                        else:
                            # round 3 is the ACT-rich round: park all the
                            # output-projection fill here
                            for ob_ in range(8):
                                fillq.extend(mk_out_thunks(ob_, 0))
                            for ob_ in range(8):
                                fillq.extend(mk_out_thunks(ob_, 1))
                        wlen = (4 * qc + 4) * 4
                        nadd = len(fillq) - pace["fi"]
                        pace["rate"] = 1.05 * nadd / wlen if qc else 5.0
                    elif kt == 0:
                        force(need.get((qc, g, 0), 0))
                    if qc == 3 and g == 1 and kt == 0:
                        # qch2 output columns read YTg written by round-2
                        # epilogues; the last lands D-1 slots into round 3,
                        # so enqueue these well after it
                        for ob_ in range(8):
                            fillq.extend(mk_out_thunks(ob_, 2))
                    q0 = qc * 512
                    gq = g * T
                    k0 = kt * 128
                    toff = 128 * (kt - 4 * qc) if kt >= 4 * qc else 0
                    off_l[(qc, g, kt)] = toff
                    diag = kt >= 4 * qc
                    sAB = pss.tile([128, 1024], F32, name="sAB", tag="sAB")
                    nc.tensor.matmul(
                        out=sAB[:, toff:512],
                        lhsT=KT[0:64, gq + k0: gq + k0 + 128],
                        rhs=QT[0:64, gq + q0 + toff: gq + q0 + 512],
                        start=True, stop=True,
                    )
                    nc.tensor.matmul(
                        out=sAB[:, 512 + toff:1024],
                        lhsT=KT[64:128, gq + k0: gq + k0 + 128],
                        rhs=QT[64:128, gq + q0 + toff: gq + q0 + 512],
                        start=True, stop=True, tile_position=(64, 0),
                    )
                    s3 = sAB.rearrange("p (h w) -> p h w", h=2, w=512)
                    trnf = trn_sb.rearrange("p (h w) -> p h w", h=2, w=256)
                    if diag:
                        # causal masking on the PE: add a -1e30 strict lower
                        # triangle (identity stationary, NEG-tri moving) into
                        # the S psum of both head halves before exp
                        nc.tensor.matmul(
                            out=s3[:, :, toff:toff + 128],
                            lhsT=idn_sb[:],
                            rhs=trnf[:, :, 128:256],
                            start=False, stop=True, skip_group_check=True,
                        )
                    if qc == 0:
                        eAB = es.tile([128, 1024], BF16, name="eAB", tag="eAB")
                        e3 = eAB.rearrange("p (h w) -> p h w", h=2, w=512)
                        nc.scalar.activation(
                            e3[:, :, toff:512], s3[:, :, toff:512],
                            AF.Exp, bias=pad_sb[:, kt:kt + 1])
                        e_l[(qc, g, kt)] = eAB
                    else:
                        # fp8 pair tile: slot j = kt parity
                        j = kt % 2
                        if j == 0:
                            e2 = es8.tile([128, 2048], F8, name="e2", tag="e2")
                        else:
                            e2 = e_l[(qc, g, kt - 1)]
                        e_l[(qc, g, kt)] = e2
                        e4 = e2.rearrange("p (j h w) -> p j h w",
                                          j=2, h=2, w=512)
                        elo = toff
                        if diag and j == 1:
                            # odd member: overwrite the 128-col causal gap
                            # with -1e30 so exp writes exact 0s there
                            nc.tensor.matmul(
                                out=s3[:, :, toff - 128:toff],
                                lhsT=idn_sb[:],
                                rhs=trnf[:, :, 0:128],
                                start=True, stop=True, skip_group_check=True,
                            )
                            elo = toff - 128
                        nc.scalar.activation(
                            e4[:, j, :, elo:512], s3[:, :, elo:512],
                            AF.Exp, bias=pad_sb[:, kt:kt + 1])
                jj = i - D
                if 0 <= jj < NEV:
                    qc, g, pv = EV[jj]
                    kmax = 4 * qc + 4
                    gq = g * T
                    if pv == 0:
                        o_l[(qc, g)] = (
                            pso.tile([65, 512], F32, name="oA", tag="o"),
                            pso.tile([65, 512], F32, name="oB", tag="o"))
                    oA, oB = o_l[(qc, g)]
                    if qc == 0:
                        toff = off_l[(qc, g, pv)]
                        vbase = pv * 520
                        eAB = e_l.pop((qc, g, pv))
                        nc.tensor.matmul(
                            out=oA[:, toff:512],
                            lhsT=Vt[:, vbase + 130 * g: vbase + 130 * g + 65],
                            rhs=eAB[:, toff:512],
                            start=(pv == 0), stop=(pv == kmax - 1),
                        )
                        nc.tensor.matmul(
                            out=oB[:, toff:512],
                            lhsT=Vt[:, vbase + 130 * g + 65:
                                    vbase + 130 * g + 130],
                            rhs=eAB[:, 512 + toff:1024],
                            start=(pv == 0), stop=(pv == kmax - 1),
                        )
                    elif pv % 2 == 1:
                        # DoubleRow AV over the k-tile pair (pv-1, pv)
                        pr = pv // 2
                        toff = off_l[(qc, g, pv - 1)]
                        e4 = e_l.pop((qc, g, pv)).rearrange(
                            "p (j h w) -> p j h w", j=2, h=2, w=512)
                        e_l.pop((qc, g, pv - 1))
                        npr = kmax // 2
                        nc.tensor.matmul(
                            out=oA[:, toff:512],
                            lhsT=Vf8[:, pr, :, 2 * g, 0:65],
                            rhs=e4[:, :, 0, toff:512],
                            start=(pr == 0), stop=(pr == npr - 1),
                            perf_mode=DR,
                        )
                        nc.tensor.matmul(
                            out=oB[:, toff:512],
                            lhsT=Vf8[:, pr, :, 2 * g + 1, 0:65],
                            rhs=e4[:, :, 1, toff:512],
                            start=(pr == 0), stop=(pr == npr - 1),
                            perf_mode=DR,
                        )
                    if pv == kmax - 1:
                        epilogue(qc, g)
                pump()
            # drain any fill left over
            while pace["fi"] < len(fillq):
                fillq[pace["fi"]]()
                pace["fi"] += 1

            # ---- tail: last chunk of the output projection ----
            # release the attention pools first so the tail gets a deep
            # psum pool: six units can then hold open accumulation groups
            # and their g0-g2 matmuls cover the final epilogue's latency
            ps1.release()
            obp.release()
            rp.release()
            so.release()
            es8.release()
            es.release()
            pso.release()
            pss.release()
            ptail = tc.alloc_tile_pool(name="ptail", bufs=8, space="PSUM")
            obt = tc.alloc_tile_pool(name="obt", bufs=4)
            pos = []
            for ob_ in range(8):
                po = ptail.tile([128, 512], F32, name="po2", tag="po2")
                pos.append(po)
                for g in range(3):
                    nc.tensor.matmul(
                        out=po[:],
                        lhsT=Wp_sb[:, g * C + ob_ * 128:
                                   g * C + ob_ * 128 + 128],
                        rhs=YTg[g][:, 3 * 512:4 * 512],
                        start=(g == 0), stop=False,
                    )
            for ob_ in range(8):
                po = pos[ob_]
                nc.tensor.matmul(
                    out=po[:],
                    lhsT=Wp_sb[:, 3 * C + ob_ * 128: 3 * C + ob_ * 128 + 128],
                    rhs=YTg[3][:, 3 * 512:4 * 512],
                    start=False, stop=True,
                )
                ob = obt.tile([128, 512], BF16, name="ob2", tag="ob2")
                nc.vector.tensor_copy(ob[:], po[:])
                q_ = (nc.sync, nc.scalar, nc.gpsimd)[ob_ % 3]
                q_.dma_start(
                    out=out_d[ob_ * 128:(ob_ + 1) * 128, 3 * 512:4 * 512],
                    in_=ob[:])
            obt.release()
            ptail.release()
            xs.release()

    nc.compile()
    return nc


def _in_maps(x, Wk, bk, Wq, bq, Wv, bv, Wp, bp, padding_mask):
    maps = []
    bf16 = ml_dtypes.bfloat16
    rows = np.arange(128)[:, None]
    cols = np.arange(128)[None, :]
    tri01 = (cols >= rows).astype(np.float32)
    trib = np.where(cols < rows, NEG, 0.0).astype(np.float32)
    blk = np.concatenate([np.full((128, 128), NEG, np.float32), trib], axis=1)
    trneg = np.concatenate([blk, blk], axis=1)
    f8 = ml_dtypes.float8_e4m3fn
    for core in range(8):
        b, half = divmod(core, 2)
        hs = slice(half * IC, (half + 1) * IC)
        xTb = np.ascontiguousarray(x[b].T)
        maps.append({
            "xT": xTb.astype(bf16),
            "xT8": xTb.astype(f8),
            "WqT8": np.ascontiguousarray(Wq[hs, :].T).astype(f8),
            "WkT8": np.ascontiguousarray(Wk[hs, :].T).astype(f8),
            "WvT8": np.ascontiguousarray(Wv[hs, :].T).astype(f8),
            "WqT": np.ascontiguousarray(Wq[hs, :].T).astype(bf16),
            "WkT": np.ascontiguousarray(Wk[hs, :].T).astype(bf16),
            "WvT": np.ascontiguousarray(Wv[hs, :].T).astype(bf16),
            "WpT": np.ascontiguousarray(Wp[:, hs].T).astype(bf16),
            "bqs": np.ascontiguousarray((bq[hs] * SCALE).reshape(4, 128).T),
            "bks": np.ascontiguousarray(bk[hs].reshape(4, 128).T),
            "bvr": bv[hs].reshape(1, IC).copy(),
            "padb": np.ascontiguousarray(
                np.where(padding_mask[b] != 0, 0.0, NEG)
                .astype(np.float32).reshape(NKT, 128).T),
            "tri01": tri01.astype(bf16),
            "idn": np.eye(128, dtype=np.float32).astype(bf16),
            "trneg": trneg.astype(bf16),
            "ones8": np.ones((128, 8), bf16),
        })
    return maps


def _run(inputs, trace=False, **kw):
    if "nc" not in _CACHE:
        _CACHE["nc"] = _build()
    nc = _CACHE["nc"]
    ins = {k: np.asarray(v, dtype=np.float32) if k != "padding_mask"
           else np.asarray(v) for k, v in inputs.items()}
    maps = _in_maps(**ins)
    res = run_bass_kernel_spmd(nc, maps, core_ids=list(range(8)), trace=trace, **kw)
    bp = np.asarray(inputs["bp"], np.float32)
    y = np.empty((B, T, C), np.float32)
    for b in range(B):
        y[b] = (res.results[2 * b]["out"].astype(np.float32)
                + res.results[2 * b + 1]["out"].astype(np.float32)).T + bp
    return y, res


def kernel(**inputs):
    y, _ = _run(inputs, trace=False)
    return y



# revision 40
# speedup vs baseline: 1.0138x; 1.0017x over previous
"""Causal self-attention TRN2 Bass kernel (fp8-DoubleRow, flat-scheduled).

Problem: B=4, T=2048, C=1024, H=16 heads (HD=64), torch-Linear semantics
(y = x @ W.T + b), causal + padding mask, softmax, output projection.

Sharding: 8 cores = (batch b in 0..3) x (head-half in 0..1). Each core
handles one batch and 8 heads (512 of the 1024 channels of QKV / of the
contraction dim of the output projection). The two half-cores of a batch
produce partial transposed outputs that the host sums and transposes
(plus bp).

Design (276us phase-interleaved baseline -> ~261us):
  - Row-stratified fp8: causality means K/V of token chunk c are only
    attended by queries >= c, so chunk 0 (tokens 0-511) runs fully bf16
    while chunks 1-3 use fp8e4m3 DoubleRow matmuls (two 128-contraction
    tiles per pass) for the QKV projections AND the AV accumulation
    (k-tile pairs; [V|1] pairs padded to 66 cols for the 16B pair-stride
    ISA rule). Early rows keep bf16 accuracy where softmax averaging is
    weak; verified rel err 3.7e-3 vs the 2e-2 gate.
  - One flat event stream over all (round, group, k-tile) QK/exp events
    with the AV lagging D=8 slots, so group/round boundaries overlap.
    Fill work (projections of the next chunk, output projection) is
    decomposed into ~1-matmul thunks paced between events; marks force
    producers to emit before their consumer events.
  - Causal masking on the PE: an identity-stationary matmul adds a
    -1e30 strict lower triangle (and pair-gap block) into the S psum
    before exp, replacing all DVE mask multiplies/memsets.
  - Output projection runs operand-swapped (Wp stationary, Y^T moving,
    transposed bf16 output, host transposes back): freshly written Y^T
    must never be an LDWEIGHTS source - the PE hoists weight loads past
    cross-engine write dependencies (observed ragged-column races).
    Same rule: V tiles drain early (v units right after q/k g0), and
    chunk-0 q/k for g0/g1 run in the serial head.
  - Rowsum via a ones-column in the V stationary ([V|1]); epilogue:
    psum row -> SBUF copy -> reciprocal_approx_fast -> partition
    broadcast -> DVE multiply (only the copy uop can shift partitions).
  - PSUM: 2x2 S banks + 3x1 O half-banks (oA/oB split so the next
    group's AV never blocks the in-order PE on the epilogue) + 1 fill.
  - Single strided DMA per input tensor (~0.7us issue each); out DMAs
    spread over the sync/scalar/gpsimd queues.
  - Engine budget: PE ~198us busy, ACT ~147us of exp (the long pole in
    rounds 1-3, ~95% saturated in round 3), DVE ~140us.
"""

import ml_dtypes
import numpy as np

import concourse.mybir as mybir
import concourse.tile as tile
from concourse import bacc
from concourse.bass_utils import run_bass_kernel_spmd

F32 = mybir.dt.float32
F32R = mybir.dt.float32r
BF16 = mybir.dt.bfloat16
F8 = mybir.dt.float8e4
DR = mybir.MatmulPerfMode.DoubleRow
AF = mybir.ActivationFunctionType
ALU = mybir.AluOpType

B, T, C, H = 4, 2048, 1024, 16
HD = C // H          # 64
IC = C // 2          # 512 channels per core (8 heads)
NKT = T // 128       # 16 k-tiles
NCT = C // 128       # 8 contraction tiles for QKV
NEG = -1.0e30
SCALE = 1.0 / np.sqrt(HD)
D = 8                # S->AV pipeline lag (in k-tile steps)

_CACHE = {}


def _build():
    nc = bacc.Bacc("TRN2", target_bir_lowering=False, debug=False)

    xT_d = nc.dram_tensor("xT", [C, T], BF16, kind="ExternalInput").ap()
    xT8_d = nc.dram_tensor("xT8", [C, T], F8, kind="ExternalInput").ap()
    WqT8_d = nc.dram_tensor("WqT8", [C, IC], F8, kind="ExternalInput").ap()
    WkT8_d = nc.dram_tensor("WkT8", [C, IC], F8, kind="ExternalInput").ap()
    WvT8_d = nc.dram_tensor("WvT8", [C, IC], F8, kind="ExternalInput").ap()
    WqT_d = nc.dram_tensor("WqT", [C, IC], BF16, kind="ExternalInput").ap()
    WkT_d = nc.dram_tensor("WkT", [C, IC], BF16, kind="ExternalInput").ap()
    WvT_d = nc.dram_tensor("WvT", [C, IC], BF16, kind="ExternalInput").ap()
    WpT_d = nc.dram_tensor("WpT", [IC, C], BF16, kind="ExternalInput").ap()
    bq_d = nc.dram_tensor("bqs", [128, 4], F32, kind="ExternalInput").ap()
    bk_d = nc.dram_tensor("bks", [128, 4], F32, kind="ExternalInput").ap()
    bv_d = nc.dram_tensor("bvr", [1, IC], F32, kind="ExternalInput").ap()
    pad_d = nc.dram_tensor("padb", [128, NKT], F32, kind="ExternalInput").ap()
    tri_d = nc.dram_tensor("tri01", [128, 128], BF16, kind="ExternalInput").ap()
    idn_d = nc.dram_tensor("idn", [128, 128], BF16, kind="ExternalInput").ap()
    trn_d = nc.dram_tensor("trneg", [128, 512], BF16, kind="ExternalInput").ap()
    ones_d = nc.dram_tensor("ones8", [128, 8], BF16, kind="ExternalInput").ap()
    out_d = nc.dram_tensor("out", [C, T], BF16, kind="ExternalOutput").ap()

    with tile.TileContext(nc) as tc:
        with tc.tile_pool(name="pp", bufs=1) as pp:
            QT = pp.tile([128, 4 * T], BF16, name="QT")
            KT = pp.tile([128, 4 * T], BF16, name="KT")
            Vt = pp.tile([128, 4 * 520], BF16, name="Vt")
            Vt8 = pp.tile([128, NKT * 528], F8, name="Vt8")
            YTg = [pp.tile([128, T], BF16, name=f"YT{i}", uniquify=False)
                   for i in range(4)]
            Wp_sb = pp.tile([128, 4 * C], BF16, name="Wp_sb")
            Wq_sb = pp.tile([128, NCT * 512], BF16, name="Wq_sb")
            Wk_sb = pp.tile([128, NCT * 512], BF16, name="Wk_sb")
            Wv_sb = pp.tile([128, NCT * 512], BF16, name="Wv_sb")
            Wq8_sb = pp.tile([128, NCT * 512], F8, name="Wq8_sb")
            Wk8_sb = pp.tile([128, NCT * 512], F8, name="Wk8_sb")
            Wv8_sb = pp.tile([128, NCT * 512], F8, name="Wv8_sb")
            bq_sb = pp.tile([128, 4], F32, name="bq_sb")
            bk_sb = pp.tile([128, 4], F32, name="bk_sb")
            bv_sb = pp.tile([1, IC], F32, name="bv_sb")
            bvb_sb = pp.tile([128, IC], F32, name="bvb_sb")
            pad_sb = pp.tile([128, NKT], F32, name="pad_sb")
            tri_sb = pp.tile([128, 128], BF16, name="tri_sb")
            idn_sb = pp.tile([128, 128], BF16, name="idn_sb")
            trn_sb = pp.tile([128, 512], BF16, name="trn_sb")
            one8_sb = pp.tile([128, 8], BF16, name="one8_sb")

            # bf16 V for the first 4 k-tiles (qc=0); fp8 k-tile-paired V
            # (pair pr holds k-tiles 2pr, 2pr+1) for the DoubleRow AV.
            Vf = Vt.rearrange("p (k h c) -> p k h c", k=4, h=8, c=65)
            # c padded to 66 so the pair-dim (j) byte stride is 528,
            # a multiple of 16 as the DoubleRow LDWEIGHTS ISA requires
            Vf8 = Vt8.rearrange("p (pr j h c) -> p pr j h c",
                                pr=NKT // 2, j=2, h=8, c=66)
            nc.gpsimd.dma_start(out=one8_sb[:], in_=ones_d)
            for kt in range(4):
                nc.vector.tensor_copy(Vf[:, kt, :, 64], one8_sb[:, :, None])
            for pr in range(NKT // 2):
                for j in range(2):
                    nc.vector.tensor_copy(Vf8[:, pr, j, :, 64],
                                          one8_sb[:, :, None])

            xs = tc.alloc_tile_pool(name="xs", bufs=3)
            pss = tc.alloc_tile_pool(name="pss", bufs=2, space="PSUM")
            pso = tc.alloc_tile_pool(name="pso", bufs=3, space="PSUM")
            es = tc.alloc_tile_pool(name="es", bufs=D + 1)
            es8 = tc.alloc_tile_pool(name="es8", bufs=6)
            so = tc.alloc_tile_pool(name="so", bufs=4)
            rp = tc.alloc_tile_pool(name="rp", bufs=2)
            obp = tc.alloc_tile_pool(name="ob", bufs=3)
            ps1 = tc.alloc_tile_pool(name="ps1", bufs=1, space="PSUM")

            xc_t = [None] * 4

            def load_x(ch):
                # chunks >= 1 arrive fp8 for DoubleRow projections
                xc = xs.tile([128, NCT * 512], F8, name="xc", tag="xc")
                t0 = ch * 512
                nc.sync.dma_start(
                    out=xc.rearrange("p (ct t) -> p ct t", ct=NCT),
                    in_=xT8_d.rearrange("(ct p) t -> p ct t",
                                        p=128)[:, :, t0:t0 + 512])
                xc_t[ch] = xc

            # ---- head: one strided DMA per tensor (issue cost ~0.7us per
            # dma_start per queue; transfers pipeline behind the matmuls)
            xc0 = xs.tile([128, NCT * 512], BF16, name="xc", tag="xc")
            xc_t[0] = xc0
            nc.sync.dma_start(
                out=xc0.rearrange("p (ct t) -> p ct t", ct=NCT),
                in_=xT_d.rearrange("(ct p) t -> p ct t", p=128)[:, :, 0:512])
            nc.scalar.dma_start(
                out=Wq_sb.rearrange("p (ct c) -> p ct c", ct=NCT),
                in_=WqT_d.rearrange("(ct p) c -> p ct c", p=128))
            nc.gpsimd.dma_start(
                out=Wk_sb.rearrange("p (ct c) -> p ct c", ct=NCT),
                in_=WkT_d.rearrange("(ct p) c -> p ct c", p=128))
            nc.sync.dma_start(
                out=Wv_sb.rearrange("p (ct c) -> p ct c", ct=NCT),
                in_=WvT_d.rearrange("(ct p) c -> p ct c", p=128))
            nc.scalar.dma_start(out=bq_sb[:], in_=bq_d)
            nc.scalar.dma_start(out=bk_sb[:], in_=bk_d)
            nc.scalar.dma_start(out=bv_sb[:], in_=bv_d)
            nc.scalar.dma_start(out=pad_sb[:], in_=pad_d)
            nc.scalar.dma_start(out=tri_sb[:], in_=tri_d)
            nc.scalar.dma_start(out=idn_sb[:], in_=idn_d)
            nc.scalar.dma_start(out=trn_sb[:], in_=trn_d)
            nc.scalar.dma_start(
                out=Wq8_sb.rearrange("p (ct c) -> p ct c", ct=NCT),
                in_=WqT8_d.rearrange("(ct p) c -> p ct c", p=128))
            nc.gpsimd.dma_start(
                out=Wk8_sb.rearrange("p (ct c) -> p ct c", ct=NCT),
                in_=WkT8_d.rearrange("(ct p) c -> p ct c", p=128))
            nc.sync.dma_start(
                out=Wv8_sb.rearrange("p (ct c) -> p ct c", ct=NCT),
                in_=WvT8_d.rearrange("(ct p) c -> p ct c", p=128))
            for g in range(4):
                nc.gpsimd.dma_start(out=Wp_sb[:, g * C:(g + 1) * C],
                                    in_=WpT_d[g * 128:(g + 1) * 128, :])
            nc.gpsimd.partition_broadcast(bvb_sb[:], bv_sb[:])
            # prefetch x one round ahead: the DMA lands long before the
            # first fill matmul reads it (an in-order-PE stall otherwise)
            load_x(1)

            # ---- thunk-granular work units ------------------------------
            # Fill work is decomposed into ~1-matmul thunks so the pacing
            # can slot ~400ns of PE work between attention steps without
            # ever delaying the next QK matmul past the ACT exp window.
            def mk_proj_thunks(ch, kind, g):
                """QKV psum group as PE thunks; the last thunk drains.
                ch0 = bf16 (2 matmuls/thunk), else fp8 DoubleRow
                (1 matmul/thunk, two 128-contraction tiles each)."""
                st = {}
                fp8 = ch != 0
                t0 = ch * 512

                def drain():
                    pj = st["pj"]
                    if kind == "q":
                        nc.vector.tensor_scalar(
                            out=QT[:, g * T + t0: g * T + t0 + 512],
                            in0=pj[:], scalar1=SCALE,
                            scalar2=bq_sb[:, g:g + 1],
                            op0=ALU.mult, op1=ALU.add)
                    elif kind == "k":
                        nc.vector.tensor_scalar(
                            out=KT[:, g * T + t0: g * T + t0 + 512],
                            in0=pj[:], scalar1=bk_sb[:, g:g + 1],
                            scalar2=None, op0=ALU.add)
                    else:
                        kt = ch * 4 + g
                        nc.vector.tensor_tensor(
                            out=Vf8[:, kt // 2, kt % 2, :, 0:64],
                            in0=pj.rearrange("p (h c) -> p h c", h=8, c=64),
                            in1=bvb_sb.rearrange("p (h c) -> p h c", h=8, c=64),
                            op=ALU.add)
                        if kt < 4:
                            nc.vector.tensor_tensor(
                                out=Vf[:, kt, :, 0:64],
                                in0=pj.rearrange("p (h c) -> p h c", h=8, c=64),
                                in1=bvb_sb.rearrange("p (h c) -> p h c",
                                                     h=8, c=64),
                                op=ALU.add)

                def th_fp8(cp):
                    def run():
                        if cp == 0:
                            st["pj"] = ps1.tile([128, 512], F32, name="pj",
                                                tag="p1ps")
                        xcf = xc_t[ch].rearrange("p (ct t) -> p ct t", ct=NCT)
                        if kind == "q" or kind == "k":
                            W = (Wq8_sb if kind == "q" else Wk8_sb).rearrange(
                                "p (ct c) -> p ct c", ct=NCT)
                            nc.tensor.matmul(
                                out=st["pj"][:],
                                lhsT=W[:, 2 * cp:2 * cp + 2,
                                       g * 128:(g + 1) * 128],
                                rhs=xcf[:, 2 * cp:2 * cp + 2, :],
                                start=(cp == 0), stop=(cp == 3),
                                perf_mode=DR)
                        else:
                            Wvf = Wv8_sb.rearrange("p (ct c) -> p ct c",
                                                   ct=NCT)
                            nc.tensor.matmul(
                                out=st["pj"][:],
                                lhsT=xcf[:, 2 * cp:2 * cp + 2,
                                         g * 128:g * 128 + 128],
                                rhs=Wvf[:, 2 * cp:2 * cp + 2, :],
                                start=(cp == 0), stop=(cp == 3),
                                perf_mode=DR)
                        if cp == 3:
                            drain()
                    return run

                def th_bf16(ci):
                    def run():
                        if ci == 0:
                            if ch == 0:
                                # head: double-buffer via the (still idle)
                                # S-psum pool so units pipeline
                                big = pss.tile([128, 1024], F32, name="sAB",
                                               tag="sAB")
                                st["pj"] = big[:, 0:512]
                            else:
                                st["pj"] = ps1.tile([128, 512], F32,
                                                    name="pj", tag="p1ps")
                        for ct in (2 * ci, 2 * ci + 1):
                            if kind == "q" or kind == "k":
                                W = Wq_sb if kind == "q" else Wk_sb
                                nc.tensor.matmul(
                                    out=st["pj"][:],
                                    lhsT=W[:, ct * 512 + g * 128:
                                           ct * 512 + (g + 1) * 128],
                                    rhs=xc_t[ch][:, ct * 512:(ct + 1) * 512],
                                    start=(ct == 0), stop=(ct == NCT - 1))
                            else:
                                nc.tensor.matmul(
                                    out=st["pj"][:],
                                    lhsT=xc_t[ch][:, ct * 512 + g * 128:
                                                  ct * 512 + g * 128 + 128],
                                    rhs=Wv_sb[:, ct * 512:(ct + 1) * 512],
                                    start=(ct == 0), stop=(ct == NCT - 1))
                        if ci == 3:
                            drain()
                    return run

                return [th_fp8(c) if fp8 else th_bf16(c) for c in range(4)]

            def mk_out_thunks(ob_, qch):
                """Swapped output-projection group as 2 PE thunks: Wp is
                the stationary operand and Y^T streams as the MOVING
                operand, so the freshly-epilogue-written YT is never an
                LDWEIGHTS source (the PE hoists weight loads ahead of
                semaphore-protected matmuls). Output lands transposed
                [oc, q]; the host transposes back."""
                st = {}

                def th(half):
                    def run():
                        if half == 0:
                            st["po"] = ps1.tile([128, 512], F32, name="po",
                                                tag="p1ps")
                        for g in (2 * half, 2 * half + 1):
                            nc.tensor.matmul(
                                out=st["po"][:],
                                lhsT=Wp_sb[:, g * C + ob_ * 128:
                                           g * C + ob_ * 128 + 128],
                                rhs=YTg[g][:, qch * 512:(qch + 1) * 512],
                                start=(g == 0), stop=(g == 3))
                        if half == 1:
                            ob = obp.tile([128, 512], BF16, name="ob",
                                          tag="ob")
                            nc.vector.tensor_copy(ob[:], st["po"][:])
                            q_ = (nc.sync, nc.scalar, nc.gpsimd)[ob_ % 3]
                            q_.dma_start(
                                out=out_d[ob_ * 128:(ob_ + 1) * 128,
                                          qch * 512:(qch + 1) * 512],
                                in_=ob[:])
                    return run

                return [th(0), th(1)]

            def proj_chunk_thunks(ch):
                """All 12 proj units of a chunk. q/k g0 first (their moving
                reads of the x tile serialize the PE past the x DMA), then
                v as early as possible: V tiles are read as LDWEIGHTS by
                the AV matmuls, and the PE does not reliably honor write
                dependencies on freshly written weight sources unless they
                are many PE-queue instructions upstream."""
                th = []
                th += mk_proj_thunks(ch, "q", 0)
                th += mk_proj_thunks(ch, "k", 0)
                for ts in range(4):
                    th += mk_proj_thunks(ch, "v", ts)
                for g in (1, 2, 3):
                    th += mk_proj_thunks(ch, "q", g)
                    th += mk_proj_thunks(ch, "k", g)
                return th

            # ---- head: proj(0) for g0/g1 and all v up front; g2/g3 ride
            # as round-0 fill (their QK consumers are 6-8 slots downstream,
            # far enough for the LDWEIGHTS distance rule)
            for th in mk_proj_thunks(0, "q", 0) + mk_proj_thunks(0, "k", 0):
                th()
            for ts in range(4):
                for th in mk_proj_thunks(0, "v", ts):
                    th()
            for th in mk_proj_thunks(0, "q", 1) + mk_proj_thunks(0, "k", 1):
                th()

            # ---- flat event stream across all rounds --------------------
            # QK/exp events for every (qc, g, kt); the AV for event j fires
            # at slot j+D, so g-group and round boundaries overlap and ACT
            # never waits out an AV drain tail.
            EV = []
            for qc in range(4):
                for g in range(4):
                    for kt in range(4 * qc + 4):
                        EV.append((qc, g, kt))
            NEV = len(EV)

            e_l = {}     # (qc, g, kt) -> e tile
            off_l = {}   # (qc, g, kt) -> toff
            o_l = {}     # (qc, g) -> oAB psum
            fillq = []
            need = {}    # (qc, g, kt) -> fillq mark that must be emitted first
            pace = {"acc": 0.0, "rate": 0.0, "fi": 0}

            def pump():
                pace["acc"] += pace["rate"]
                while pace["acc"] >= 1.0 and pace["fi"] < len(fillq):
                    fillq[pace["fi"]]()
                    pace["fi"] += 1
                    pace["acc"] -= 1.0

            def force(mark):
                # producers must be EMITTED before their consumer events, or
                # the dependency tracker never sees the ordering (race)
                while pace["fi"] < mark:
                    fillq[pace["fi"]]()
                    pace["fi"] += 1

            def epilogue(qc, g):
                # normalize by rowsum (row 64 of each half), write Y^T.
                # The rowsum hop through rs_ is a DVE copy: only the copy
                # uop supports the partition-64 -> partition-0 move.
                q0 = qc * 512
                oA, oB = o_l.pop((qc, g))
                rs_ = rp.tile([1, 1024], F32, name="rs_", tag="rs_")
                rr_ = rp.tile([1, 1024], F32, name="rr_", tag="rr_")
                rbA = rp.tile([64, 512], F32, name="rbA", tag="rbA")
                rbB = rp.tile([64, 512], F32, name="rbB", tag="rbB")
                nc.vector.tensor_copy(rs_[:, 0:512], oA[64:65, :])
                nc.vector.reciprocal_approx_fast(rr_[:, 0:512], rs_[:, 0:512])
                nc.gpsimd.partition_broadcast(rbA[:], rr_[:, 0:512])
                nc.vector.tensor_copy(rs_[:, 512:1024], oB[64:65, :])
                nc.vector.tensor_mul(
                    YTg[g][0:64, q0: q0 + 512], oA[0:64, :], rbA[:])
                nc.vector.reciprocal_approx_fast(rr_[:, 512:1024],
                                                 rs_[:, 512:1024])
                nc.gpsimd.partition_broadcast(rbB[:], rr_[:, 512:1024])
                nc.vector.tensor_mul(
                    YTg[g][64:128, q0: q0 + 512], oB[0:64, :], rbB[:])

            for i in range(NEV + D):
                if i < NEV:
                    qc, g, kt = EV[i]
                    if g == 0 and kt == 0:
                        # round head: emit everything this round's attention
                        # reads, then enqueue the next round's fill
                        force(need.get((qc, 0, 0), 0))
                        if qc == 0:
                            for gg in (2, 3):
                                fillq.extend(mk_proj_thunks(0, "q", gg))
                                fillq.extend(mk_proj_thunks(0, "k", gg))
                                need[(0, gg, 0)] = len(fillq)
                            load_x(2)
                            fillq.extend(proj_chunk_thunks(1))
                            need[(1, 0, 0)] = len(fillq)
                        elif qc < 3:
                            if qc == 1:
                                load_x(3)
                            fillq.extend(proj_chunk_thunks(qc + 1))
                            need[(qc + 1, 0, 0)] = len(fillq)
                        else:
                            # round 3 is the ACT-rich round: park all the
                            # output-projection fill here
                            for ob_ in range(8):
                                fillq.extend(mk_out_thunks(ob_, 0))
                            for ob_ in range(8):
                                fillq.extend(mk_out_thunks(ob_, 1))
                        wlen = (4 * qc + 4) * 4
                        nadd = len(fillq) - pace["fi"]
                        pace["rate"] = 1.05 * nadd / wlen if qc else 5.0
                    elif kt == 0:
                        force(need.get((qc, g, 0), 0))
                    if qc == 3 and g == 1 and kt == 0:
                        # qch2 output columns read YTg written by round-2
                        # epilogues; the last lands D-1 slots into round 3,
                        # so enqueue these well after it
                        for ob_ in range(8):
                            fillq.extend(mk_out_thunks(ob_, 2))
                    q0 = qc * 512
                    gq = g * T
                    k0 = kt * 128
                    toff = 128 * (kt - 4 * qc) if kt >= 4 * qc else 0
                    off_l[(qc, g, kt)] = toff
                    diag = kt >= 4 * qc
                    sAB = pss.tile([128, 1024], F32, name="sAB", tag="sAB")
                    nc.tensor.matmul(
                        out=sAB[:, toff:512],
                        lhsT=KT[0:64, gq + k0: gq + k0 + 128],
                        rhs=QT[0:64, gq + q0 + toff: gq + q0 + 512],
                        start=True, stop=True,
                    )
                    nc.tensor.matmul(
                        out=sAB[:, 512 + toff:1024],
                        lhsT=KT[64:128, gq + k0: gq + k0 + 128],
                        rhs=QT[64:128, gq + q0 + toff: gq + q0 + 512],
                        start=True, stop=True, tile_position=(64, 0),
                    )
                    s3 = sAB.rearrange("p (h w) -> p h w", h=2, w=512)
                    trnf = trn_sb.rearrange("p (h w) -> p h w", h=2, w=256)
                    if diag:
                        # causal masking on the PE: add a -1e30 strict lower
                        # triangle (identity stationary, NEG-tri moving) into
                        # the S psum of both head halves before exp
                        nc.tensor.matmul(
                            out=s3[:, :, toff:toff + 128],
                            lhsT=idn_sb[:],
                            rhs=trnf[:, :, 128:256],
                            start=False, stop=True, skip_group_check=True,
                        )
                    if qc == 0:
                        eAB = es.tile([128, 1024], BF16, name="eAB", tag="eAB")
                        e3 = eAB.rearrange("p (h w) -> p h w", h=2, w=512)
                        nc.scalar.activation(
                            e3[:, :, toff:512], s3[:, :, toff:512],
                            AF.Exp, bias=pad_sb[:, kt:kt + 1])
                        e_l[(qc, g, kt)] = eAB
                    else:
                        # fp8 pair tile: slot j = kt parity
                        j = kt % 2
                        if j == 0:
                            e2 = es8.tile([128, 2048], F8, name="e2", tag="e2")
                        else:
                            e2 = e_l[(qc, g, kt - 1)]
                        e_l[(qc, g, kt)] = e2
                        e4 = e2.rearrange("p (j h w) -> p j h w",
                                          j=2, h=2, w=512)
                        elo = toff
                        if diag and j == 1:
                            # odd member: overwrite the 128-col causal gap
                            # with -1e30 so exp writes exact 0s there
                            nc.tensor.matmul(
                                out=s3[:, :, toff - 128:toff],
                                lhsT=idn_sb[:],
                                rhs=trnf[:, :, 0:128],
                                start=True, stop=True, skip_group_check=True,
                            )
                            elo = toff - 128
                        nc.scalar.activation(
                            e4[:, j, :, elo:512], s3[:, :, elo:512],
                            AF.Exp, bias=pad_sb[:, kt:kt + 1])
                jj = i - D
                if 0 <= jj < NEV:
                    qc, g, pv = EV[jj]
                    kmax = 4 * qc + 4
                    gq = g * T
                    if pv == 0:
                        o_l[(qc, g)] = (
                            pso.tile([65, 512], F32, name="oA", tag="o"),
                            pso.tile([65, 512], F32, name="oB", tag="o"))
                    oA, oB = o_l[(qc, g)]
                    if qc == 0:
                        toff = off_l[(qc, g, pv)]
                        vbase = pv * 520
                        eAB = e_l.pop((qc, g, pv))
                        nc.tensor.matmul(
                            out=oA[:, toff:512],
                            lhsT=Vt[:, vbase + 130 * g: vbase + 130 * g + 65],
                            rhs=eAB[:, toff:512],
                            start=(pv == 0), stop=(pv == kmax - 1),
                        )
                        nc.tensor.matmul(
                            out=oB[:, toff:512],
                            lhsT=Vt[:, vbase + 130 * g + 65:
                                    vbase + 130 * g + 130],
                            rhs=eAB[:, 512 + toff:1024],
                            start=(pv == 0), stop=(pv == kmax - 1),
                        )
                    elif pv % 2 == 1:
                        # DoubleRow AV over the k-tile pair (pv-1, pv)
                        pr = pv // 2
                        toff = off_l[(qc, g, pv - 1)]
                        e4 = e_l.pop((qc, g, pv)).rearrange(
                            "p (j h w) -> p j h w", j=2, h=2, w=512)
                        e_l.pop((qc, g, pv - 1))
                        npr = kmax // 2
                        nc.tensor.matmul(
                            out=oA[:, toff:512],
                            lhsT=Vf8[:, pr, :, 2 * g, 0:65],
                            rhs=e4[:, :, 0, toff:512],
                            start=(pr == 0), stop=(pr == npr - 1),
                            perf_mode=DR,
                        )
                        nc.tensor.matmul(
                            out=oB[:, toff:512],
                            lhsT=Vf8[:, pr, :, 2 * g + 1, 0:65],
                            rhs=e4[:, :, 1, toff:512],
                            start=(pr == 0), stop=(pr == npr - 1),
                            perf_mode=DR,
                        )
                    if pv == kmax - 1:
                        epilogue(qc, g)
                pump()
            # drain any fill left over
            while pace["fi"] < len(fillq):
                fillq[pace["fi"]]()
                pace["fi"] += 1

            # ---- tail: last chunk of the output projection ----
            # release the attention pools first so the tail gets a deep
            # psum pool: six units can then hold open accumulation groups
            # and their g0-g2 matmuls cover the final epilogue's latency
            ps1.release()
            obp.release()
            rp.release()
            so.release()
            es8.release()
            es.release()
            pso.release()
            pss.release()
            ptail = tc.alloc_tile_pool(name="ptail", bufs=8, space="PSUM")
            obt = tc.alloc_tile_pool(name="obt", bufs=4)
            pos = []
            for ob_ in range(8):
                po = ptail.tile([128, 512], F32, name="po2", tag="po2")
                pos.append(po)
                for g in range(3):
                    nc.tensor.matmul(
                        out=po[:],
                        lhsT=Wp_sb[:, g * C + ob_ * 128:
                                   g * C + ob_ * 128 + 128],
                        rhs=YTg[g][:, 3 * 512:4 * 512],
                        start=(g == 0), stop=False,
                    )
            for ob_ in range(8):
                po = pos[ob_]
                nc.tensor.matmul(
                    out=po[:],
                    lhsT=Wp_sb[:, 3 * C + ob_ * 128: 3 * C + ob_ * 128 + 128],
                    rhs=YTg[3][:, 3 * 512:4 * 512],
                    start=False, stop=True,
                )
                ob = obt.tile([128, 512], BF16, name="ob2", tag="ob2")
                nc.vector.tensor_copy(ob[:], po[:])
                q_ = (nc.sync, nc.scalar, nc.gpsimd)[ob_ % 3]
                q_.dma_start(
                    out=out_d[ob_ * 128:(ob_ + 1) * 128, 3 * 512:4 * 512],
                    in_=ob[:])
            obt.release()
            ptail.release()
            xs.release()

    nc.compile()
    return nc


def _in_maps(x, Wk, bk, Wq, bq, Wv, bv, Wp, bp, padding_mask):
    maps = []
    bf16 = ml_dtypes.bfloat16
    rows = np.arange(128)[:, None]
    cols = np.arange(128)[None, :]
    tri01 = (cols >= rows).astype(np.float32)
    trib = np.where(cols < rows, NEG, 0.0).astype(np.float32)
    blk = np.concatenate([np.full((128, 128), NEG, np.float32), trib], axis=1)
    trneg = np.concatenate([blk, blk], axis=1)
    f8 = ml_dtypes.float8_e4m3fn
    for core in range(8):
        b, half = divmod(core, 2)
        hs = slice(half * IC, (half + 1) * IC)
        xTb = np.ascontiguousarray(x[b].T)
        maps.append({
            "xT": xTb.astype(bf16),
            "xT8": xTb.astype(f8),
            "WqT8": np.ascontiguousarray(Wq[hs, :].T).astype(f8),
            "WkT8": np.ascontiguousarray(Wk[hs, :].T).astype(f8),
            "WvT8": np.ascontiguousarray(Wv[hs, :].T).astype(f8),
            "WqT": np.ascontiguousarray(Wq[hs, :].T).astype(bf16),
            "WkT": np.ascontiguousarray(Wk[hs, :].T).astype(bf16),
            "WvT": np.ascontiguousarray(Wv[hs, :].T).astype(bf16),
            "WpT": np.ascontiguousarray(Wp[:, hs].T).astype(bf16),
            "bqs": np.ascontiguousarray((bq[hs] * SCALE).reshape(4, 128).T),
            "bks": np.ascontiguousarray(bk[hs].reshape(4, 128).T),
            "bvr": bv[hs].reshape(1, IC).copy(),
            "padb": np.ascontiguousarray(
                np.where(padding_mask[b] != 0, 0.0, NEG)
                .astype(np.float32).reshape(NKT, 128).T),
            "tri01": tri01.astype(bf16),
            "idn": np.eye(128, dtype=np.float32).astype(bf16),
            "trneg": trneg.astype(bf16),
            "ones8": np.ones((128, 8), bf16),
        })
    return maps


def _run(inputs, trace=False, **kw):
    if "nc" not in _CACHE:
        _CACHE["nc"] = _build()
    nc = _CACHE["nc"]
    ins = {k: np.asarray(v, dtype=np.float32) if k != "padding_mask"
           else np.asarray(v) for k, v in inputs.items()}
    maps = _in_maps(**ins)
    res = run_bass_kernel_spmd(nc, maps, core_ids=list(range(8)), trace=trace, **kw)
    bp = np.asarray(inputs["bp"], np.float32)
    y = np.empty((B, T, C), np.float32)
    for b in range(B):
        y[b] = (res.results[2 * b]["out"].astype(np.float32)
                + res.results[2 * b + 1]["out"].astype(np.float32)).T + bp
    return y, res


def kernel(**inputs):
    y, _ = _run(inputs, trace=False)
    return y

